# revision 7
# baseline (speedup 1.0000x reference)
"""AmplitudeEncoder Trainium2 kernel (v6: bf16 stream, inv2-scaled masks).

Computes, for x [64, 784] f32:
    state = pad(x, [.., 1001]); state /= ||state||_2 (per row)
    out[b] = outer(state[b], state[b])  -> [64, 1001, 1001] f32

Pure data-parallel across 8 NeuronCores: batch sharded 8 samples/core.

Structural facts exploited:
  * state[784:] == 0 -> out[b] nonzero only in the top-left [784, 784]
    block; only that block is computed/written (host fills the zeros).
  * rel-err gate is 2e-2; bf16 output (~3e-3 rel err) halves the HBM
    write stream to 9.83 MB/core; the kernel is output-DMA bound at
    ~400 B/ns (16 SDMA engines x ~25 B/ns).
  * out[i,j] = (x_i/||x||^2) * x_j: 1/||x||^2 is folded into the PE
    broadcast masks (one tensor_scalar over the [8,1024] mask tile), so
    prow arrives in PSUM already normalized; evacuation is a plain
    bf16 copy and the column factors are RAW x.
  * the column factors x[b, c*128+p] are just a layout transform of the
    input, so the host ships the shard twice: row-major (x) and
    partition-major (xt[p, c, b]) - no on-device transposes at all.

Per-core dataflow:
  startup: x -> [8,1024] tile; xt -> [128, 7, 8] col tile (scalar ring,
           in parallel with x on the sync ring). ACT casts xb = bf16(x).
           DVE: ssq via scalar_tensor_tensor accum -> reciprocal ->
           masks_s = masks * inv2 (per-partition scalar, 4x mode).
  per sample b:
    PE:    prow_b = masks_s_b.T @ xb -> PSUM f32 [128, 784] normalized
           row bcast (2 bf16 matmuls, psum-bank split).
    ACT:   rowb = bf16(prow_b) -> SBUF (plain Copy);
           chunk6 tail: c6 = rowb[:16] * xt[:16, 6, b] (own tile, DMA
           issued on the scalar HWDGE ring to keep sync unclogged).
    DVE:   6 chunk products ot[:, c, :] = rowb * xt[:, c, b]
           (tensor_scalar, 4x mode) into a [128, 6, 784] bf16 tile.
    DMA:   sync ring: [128, 6*784] dense + scalar ring: [16, 784] tail.
           Sample 0: chunk 0 computed straight from PSUM (1x) in two
           half-width pieces and DMA'd alone for earliest first bytes.
  scratch: scr[b, p, c, f] = out[b, c*128+p, f] (dense bf16; each
           partition line is 9408 B contiguous in HBM). Host transposes
           (c,p)->rows, casts to f32, pads zeros.
"""

import numpy as np
import ml_dtypes

import concourse.bacc as bacc
import concourse.tile as tile
from concourse import mybir
from concourse.bass_utils import run_bass_kernel_spmd

N_CORES = 8
B = 64  # full batch
F = 784  # features per sample
D = 1001  # statevector dim (comb(14, 4))
P = 128  # SBUF partitions
NCHUNK = 7  # output row chunks (6 full + 16-row tail)
DP = 1024  # padded feature length
BSH = B // N_CORES  # samples per core
R6 = F - 6 * P  # 16 nonzero rows in the last chunk
HF = 392  # half chunk width for sample 0's first piece

F32 = mybir.dt.float32
BF16 = mybir.dt.bfloat16

_compiled_nc = None


def _masks() -> np.ndarray:
    """[8, 1024] bf16 per-sample broadcast masks (row b of slice b all-ones)."""
    m = np.zeros((BSH, BSH, P), dtype=np.float32)
    for b in range(BSH):
        m[b, b, :] = 1.0
    return m.reshape(BSH, BSH * P).astype(ml_dtypes.bfloat16)


def _build():
    nc = bacc.Bacc("TRN2", debug=False)
    x = nc.dram_tensor("x", [BSH, F], F32, kind="ExternalInput")
    # xt[p, c, b] = x[b, c*128+p]: host-transposed column factors
    xtd = nc.dram_tensor("xt", [P, NCHUNK, BSH], F32, kind="ExternalInput")
    masksd = nc.dram_tensor("masks", [BSH, BSH * P], BF16, kind="ExternalInput")
    # dense scratch: scr[b, p, c, f] = out[b, c*128+p, f]
    scr = nc.dram_tensor("scr", [BSH, P, NCHUNK, F], BF16, kind="ExternalOutput")

    with tile.TileContext(nc) as tc:
        with (
            tc.tile_pool(name="small", bufs=1) as small,
            tc.tile_pool(name="prow", bufs=2, space="PSUM") as prowp,
            tc.tile_pool(name="rowb", bufs=4) as rowbp,
            tc.tile_pool(name="ot", bufs=8) as otp,
            tc.tile_pool(name="c6", bufs=8) as c6p,
        ):
            # ---- inputs: x + masks on sync, xt on scalar (parallel issue)
            xp_t = small.tile([BSH, DP], F32)
            xt_t = small.tile([P, NCHUNK, BSH], F32)
            nc.scalar.dma_start(xt_t[:], xtd.ap())
            dummy = small.tile([BSH, 1], F32)
            nc.scalar.mul(dummy[:], xp_t[:, F : F + 1], 1.0)  # ACT table preload
            nc.sync.dma_start(xp_t[:, :F], x.ap())
            masks_t = small.tile([BSH, BSH * P], BF16)
            nc.sync.dma_start(masks_t[:], masksd.ap())

            # ---- raw x cast for the PE row broadcasts (ACT, off DVE chain)
            xb_t = small.tile([BSH, DP], BF16)
            nc.scalar.copy(xb_t[:, :F], xp_t[:, :F])

            # ---- inv2 = 1/sum(x^2), folded into the broadcast masks
            sq_t = small.tile([BSH, F], F32)
            ssq = small.tile([BSH, 1], F32)
            nc.vector.scalar_tensor_tensor(
                sq_t[:],
                xp_t[:, :F],
                1.0,
                xp_t[:, :F],
                mybir.AluOpType.mult,
                mybir.AluOpType.mult,
                accum_out=ssq[:],
            )
            inv2 = small.tile([BSH, 1], F32)
            nc.vector.reciprocal(inv2[:], ssq[:])
            masks_s = small.tile([BSH, BSH * P], BF16)
            nc.vector.tensor_scalar_mul(masks_s[:], masks_t[:], inv2[:])

            def emit_prow(b):
                prow = prowp.tile([P, DP], F32, tag="prow")
                nc.tensor.matmul(
                    prow[:, :512],
                    lhsT=masks_s[:, b * P : (b + 1) * P],
                    rhs=xb_t[:, :512],
                    start=True,
                    stop=True,
                )
                nc.tensor.matmul(
                    prow[:, 512:F],
                    lhsT=masks_s[:, b * P : (b + 1) * P],
                    rhs=xb_t[:, 512:F],
                    start=True,
                    stop=True,
                )
                return prow

            # ---- per sample: PE bcast -> ACT evac -> DVE chunks -> DMA
            for b in range(BSH):
                prow = emit_prow(b)
                ot = otp.tile([P, 6, F], BF16, tag="ot")
                if b == 0:
                    # chunk 0 straight from PSUM (1x), in halves, for
                    # earliest first output bytes
                    nc.vector.tensor_scalar_mul(
                        ot[:, 0, :HF], prow[:, :HF], xt_t[:, 0, 0:1]
                    )
                    nc.sync.dma_start(scr.ap()[b, :, 0, :HF], ot[:, 0, :HF])
                    nc.vector.tensor_scalar_mul(
                        ot[:, 0, HF:], prow[:, HF:F], xt_t[:, 0, 0:1]
                    )
                    nc.sync.dma_start(scr.ap()[b, :, 0, HF:], ot[:, 0, HF:])
                rowb = rowbp.tile([P, F], BF16, tag="rowb")
                nc.scalar.copy(rowb[:], prow[:, :F])
                if b == 0:
                    for c in range(1, 4):
                        nc.vector.tensor_scalar_mul(
                            ot[:, c, :], rowb[:], xt_t[:, c, b : b + 1]
                        )
                    nc.sync.dma_start(scr.ap()[b, :, 1:4, :], ot[:, 1:4, :])
                    for c in range(4, 6):
                        nc.vector.tensor_scalar_mul(
                            ot[:, c, :], rowb[:], xt_t[:, c, b : b + 1]
                        )
                    nc.sync.dma_start(scr.ap()[b, :, 4:6, :], ot[:, 4:6, :])
                else:
                    for c in range(6):
                        nc.vector.tensor_scalar_mul(
                            ot[:, c, :], rowb[:], xt_t[:, c, b : b + 1]
                        )
                    nc.sync.dma_start(scr.ap()[b, :, 0:6, :], ot[:])
                c6 = c6p.tile([R6, F], BF16, tag="c6")
                nc.scalar.mul(c6[:], rowb[:R6, :], xt_t[:R6, 6, b : b + 1])
                nc.scalar.dma_start(scr.ap()[b, :R6, 6, :], c6[:])

    nc.compile()
    return nc


def _get_nc():
    global _compiled_nc
    if _compiled_nc is None:
        _compiled_nc = _build()
    return _compiled_nc


def run_sharded(x: np.ndarray, trace: bool = False):
    """Run the SPMD kernel; returns (full_output, BassKernelResults)."""
    x = np.ascontiguousarray(np.asarray(x, dtype=np.float32))
    assert x.shape == (B, F), x.shape
    nc = _get_nc()
    masks = _masks()
    in_maps = []
    for i in range(N_CORES):
        xs = x[i * BSH : (i + 1) * BSH]
        # xt[p, c, b] = x[b, c*128+p] (pad rows 784..895 with zeros)
        xtp = np.zeros((BSH, NCHUNK * P), dtype=np.float32)
        xtp[:, :F] = xs
        xt = np.ascontiguousarray(xtp.reshape(BSH, NCHUNK, P).transpose(2, 1, 0))
        in_maps.append({"x": xs, "xt": xt, "masks": masks})
    res = run_bass_kernel_spmd(nc, in_maps, core_ids=list(range(N_CORES)), trace=trace)
    out = np.zeros((B, D, D), dtype=np.float32)
    for i in range(N_CORES):
        blk = np.asarray(res.results[i]["scr"]).astype(np.float32)
        # scr[b, p, c, f] -> rows r = c*128+p
        rows = blk.transpose(0, 2, 1, 3).reshape(BSH, NCHUNK * P, F)[:, :F, :]
        out[i * BSH : (i + 1) * BSH, :F, :F] = rows
    return out, res


def kernel(x: np.ndarray) -> np.ndarray:
    out, _ = run_sharded(x)
    return out
